# revision 3
# baseline (speedup 1.0000x reference)
"""Local sparse (banded) attention v2 for Trainium2, 8 NeuronCores.

Problem: B=2, H=12, L=4096, D=64, window=128 (position i attends to [i-128, i+128]).

v2 changes vs the 69.7us baseline (all hw-probe-validated in probe.py):
- Band mask applied IN PSUM via accumulating identity-weight matmuls that add
  -384 to out-of-band score cells before exp. exp then underflows masked
  cells to exactly 0 (ACT path) / -0.0 (DVE saturating int16 path), removing
  every mask multiply from DVE (~18us of DVE work in the baseline).
- QK matmuls in fp8e4 DoubleRow perf mode (contraction 64 = 2x32): 0.5
  cycles/row on PE, halving QK cost. q/k inputs are cast to fp8 on host.
- exp split between ACT (exact Exp activation) and DVE (Schraudolph int16
  fast-exp) at half-tile (384-col) granularity, fraction DVE_NUM/DVE_DEN.
- Edge strips (j=0/31) only compute/exp their valid 256 cols.
- PV + normalize unchanged: fp16 P^T x V_aug matmuls with a ones-column
  producing the softmax denominator; DVE reciprocal + broadcast multiply.
"""

import contextlib
import os
import sys

sys.path.insert(0, "/opt/trn_rl_repo")
os.environ.setdefault("JAX_PLATFORMS", "axon")

import numpy as np
import ml_dtypes

import concourse.bass as bass
import concourse.mybir as mybir
from concourse import tile

B, H, L, D = 2, 12, 4096, 64
W = 128
NBLK = L // 128  # 32
HPC = 3          # heads per core
NCORES = 8
F16 = mybir.dt.float16
F32 = mybir.dt.float32
F8 = mybir.dt.float8e4
I16 = mybir.dt.int16
EXP = mybir.ActivationFunctionType.Exp
DR = mybir.MatmulPerfMode.DoubleRow

BIAS_B = 384.0  # mask bias; -384 in e4m3 exactly; kills exp on both paths

# Schraudolph fast-exp in fp16 bit-space: exp(s/8) ~= bitcast_f16(int16(
# s * (0.125 * 2^10 / ln2) + (15 * 2^10 - 44.5))), max rel err ~3.0%.
# With the -384 mask bias the convert saturates to -32768 = -0.0 (verified
# on hw in probe.py; CoreSim wraps instead - hw is truth).
FEXP_A = 0.125 * 1024.0 / float(np.log(2.0))
FEXP_B = 15.0 * 1024.0 - 44.5

# fraction of exp half-tiles computed on DVE (Schraudolph) instead of ACT
DVE_NUM, DVE_DEN = 0, 1

PSS_BUFS = 3
DMA_CHUNK = 4
DROP: set = set()


def _rel_slice(i: int, j: int) -> int:
    if j == 0:
        return i
    if j == NBLK - 1:
        return i - (NBLK - 3)
    return i - j + 1


_NO_SPLIT_OPCODES = {"AllEngineBarrier", "Halt", "Call", "Branch",
                     "CompareAndBranch", "IndirectBranch", "BranchHint"}


def _legalize_matmul_waits(nc: bass.Bass) -> None:
    """TPB engine instructions encode a single sync wait; walrus refuses
    more. Split extras onto NoOps (one wait each) inserted right before the
    instruction on the same engine queue."""
    f = nc.m.functions[0]
    for blk in f.blocks:
        il = blk.instructions
        idx = 0
        while idx < len(il):
            inst = il[idx]
            si = inst.sync_info
            if (
                si is not None
                and len(si.on_wait) > 1
                and inst.opcode not in _NO_SPLIT_OPCODES
            ):
                waits = list(si.on_wait)
                for w_i, w in enumerate(waits[:-1]):
                    nop = mybir.InstNoOp(name=f"{inst.name}-wnop{w_i}")
                    nop.engine = inst.engine
                    nop.sync_info = mybir.SyncInfo(on_wait=[w], on_update=[])
                    nc.register_instruction(nop)
                    il.insert(idx, nop)
                    idx += 1
                inst.sync_info = mybir.SyncInfo(
                    on_wait=waits[-1:], on_update=list(si.on_update)
                )
            idx += 1


def _exp_window(j: int) -> tuple[int, int]:
    """Valid column window of key-block j's 384-wide query strip."""
    if j == 0:
        return 0, 256
    if j == NBLK - 1:
        return 128, 384
    return 0, 384


def build_nc(n_heads: int = HPC, repeat: int = 1, loop: int = 0) -> bass.Bass:
    nc = bass.Bass("TRN2", target_bir_lowering=False, debug=False)
    qT = nc.dram_tensor("qT", [n_heads, 64, L], F16, kind="ExternalInput").ap()
    kT = nc.dram_tensor("kT", [n_heads, 64, L], F16, kind="ExternalInput").ap()
    vA = nc.dram_tensor("vA", [n_heads, 128, NBLK, 65], F16, kind="ExternalInput").ap()
    # mask-bias constants (fp8): W8 identity (DoubleRow layout), interior and
    # edge bias patterns
    w8 = nc.dram_tensor("w8", [128, 128], F16, kind="ExternalInput").ap()
    bInt = nc.dram_tensor("bInt", [128, 512], F16, kind="ExternalInput").ap()
    bEdge = nc.dram_tensor("bEdge", [128, 512], F16, kind="ExternalInput").ap()
    # partition-major out layout: each partition line is one contiguous 4KB
    # DMA descriptor (vs 128B descriptors for [NBLK, 128, 64])
    out = nc.dram_tensor("out", [n_heads, 128, NBLK, 64], F16, kind="ExternalOutput").ap()

    with tile.TileContext(nc) as tc:
        with (
            tc.tile_pool(name="cst", bufs=1) as cst,
            tc.tile_pool(name="io", bufs=2) as io,
            tc.tile_pool(name="ptp", bufs=2) as ptp,
            tc.tile_pool(name="pss", bufs=PSS_BUFS, space="PSUM") as pss,
            tc.tile_pool(name="pso", bufs=2, space="PSUM") as pso,
        ):
            ident = cst.tile([128, 128], F16, name="ident")
            bias_int = cst.tile([128, 512], F16, name="bias_int")
            bias_edge = cst.tile([128, 512], F16, name="bias_edge")
            nc.scalar.dma_start(out=ident, in_=w8)
            nc.scalar.dma_start(out=bias_int, in_=bInt)
            nc.scalar.dma_start(out=bias_edge, in_=bEdge)

            n_pass = repeat * n_heads
            pairs = []
            rh = 0
            while rh < n_pass:
                if rh + 1 < n_pass:
                    pairs.append((rh, rh + 1)); rh += 2
                else:
                    pairs.append((rh,)); rh += 1
            loop_cm = tc.For_i(0, loop) if loop else contextlib.nullcontext()
            with loop_cm:
                _run_passes(nc, tc, pairs, n_heads, cst, io, ptp, pss, pso,
                            qT, kT, vA, out, ident, bias_int, bias_edge)
    _legalize_matmul_waits(nc)
    return nc


def _run_passes(nc, tc, pairs, n_heads, cst, io, ptp, pss, pso,
                qT, kT, vA, out, ident, bias_int, bias_edge):
    half_ctr = [0]

    def on_dve() -> bool:
        h = half_ctr[0]
        half_ctr[0] += 1
        return (h * DVE_NUM) % DVE_DEN < DVE_NUM

    for pr, hds in enumerate(pairs):
        nh = len(hds)
        qt = io.tile([128, L], F16, tag="qt", name=f"qt{pr}")
        kt = io.tile([128, L], F16, tag="kt", name=f"kt{pr}")
        vts = []
        CL = L // 4       # first-chunk cols (q/k)
        CN = NBLK // 4    # first-chunk blocks (v)
        h0 = hds[0] % n_heads
        # A-chunks first (enable the first QK/PV quickly), then B-remainders.
        # For pairs, both heads are adjacent in DRAM: one DMA covers both
        # (out spans 128 partitions), halving HWDGE descriptor-gen serials.
        for u, rhh in enumerate(hds):
            h = rhh % n_heads
            nc.sync.dma_start(out=kt[u * 64:(u + 1) * 64, 0:CL],
                              in_=kT[h][:, 0:CL])
            nc.sync.dma_start(out=qt[u * 64:(u + 1) * 64, 0:CL],
                              in_=qT[h][:, 0:CL])
        for u, rhh in enumerate(hds):
            h = rhh % n_heads
            vt = io.tile([128, NBLK * 65], F16, tag=f"vt{u}", name=f"vt{pr}_{u}")
            vtv = vt.rearrange("p (n c) -> p n c", n=NBLK)
            nc.sync.dma_start(out=vtv[:, 0:CN, :], in_=vA[h][:, 0:CN, :])
            vts.append(vt)
        for u, rhh in enumerate(hds):
            h = rhh % n_heads
            nc.sync.dma_start(out=kt[u * 64:(u + 1) * 64, CL:L],
                              in_=kT[h][:, CL:L])
            nc.sync.dma_start(out=qt[u * 64:(u + 1) * 64, CL:L],
                              in_=qT[h][:, CL:L])
        for u, rhh in enumerate(hds):
            h = rhh % n_heads
            nc.sync.dma_start(
                out=vts[u].rearrange("p (n c) -> p n c", n=NBLK)[:, CN:NBLK, :],
                in_=vA[h][:, CN:NBLK, :])
        # single-wait funnel touches (see _legalize_matmul_waits); DVE so the
        # ACT queue stays pure-Exp (no activation-table swaps on hw)
        qs = cst.tile([128, 2], F16, tag="qs", name=f"qs{pr}", bufs=1)
        ks = cst.tile([128, 2], F16, tag="ks", name=f"ks{pr}", bufs=1)
        qtc = qt.rearrange("p (c x) -> p c x", c=4)
        ktc = kt.rearrange("p (c x) -> p c x", c=4)
        nc.vector.tensor_copy(qs[0:64 * nh, :], qtc[0:64 * nh, 0:2:1, 0:1])
        nc.vector.tensor_copy(ks[0:64 * nh, :], ktc[0:64 * nh, 0:2:1, 0:1])
        for u in range(nh):
            vs = cst.tile([128, 1], F16, tag=f"vs{u}", name=f"vs{pr}_{u}", bufs=1)
            nc.vector.tensor_copy(vs, vts[u][:, 0:1])

        pt = ptp.tile([128, NBLK * nh * 384], F16, tag="pt", name=f"pt{pr}")
        if "exp" in DROP:
            nc.vector.memset(pt[:, 0:1], 0.0)
        ots = [io.tile([128, NBLK * 64], F16, tag=f"ot{u}", name=f"ot{pr}_{u}")
               for u in range(nh)]
        if "pv" in DROP:
            for u in range(nh):
                nc.vector.memset(ots[u][:, 0:1], 0.0)
        rts = [io.tile([128, NBLK], F32, tag=f"rt{u}", name=f"rt{pr}_{u}")
               for u in range(nh)]

        s2_hold = None
        for j in range(NBLK):
            st = 0 if j == 0 else (L - 384 if j == NBLK - 1 else (j - 1) * 128)
            lo, hi = _exp_window(j)
            if nh == 2:
                s2 = pss.tile([128, 1024], F32, tag="s2", name=f"s2_{pr}_{j}")
                s2v = s2.rearrange("p (u s c) -> p u s c", u=2, s=4)
                # QK first (start=True marks each head's bank pending-zero),
                # then the mask-bias accumulates onto written cells only
                for u in range(nh):
                    if "qk" in DROP:
                        continue
                    nc.tensor.matmul(
                        s2[:, u * 512 + lo : u * 512 + hi],
                        lhsT=kt[u * 64:(u + 1) * 64, j * 128:(j + 1) * 128],
                        rhs=qt[u * 64:(u + 1) * 64, st + lo : st + hi],
                        start=True,
                        stop=False,
                        tile_position=(u * 64, 0),
                        skip_group_check=True)
                if "mask" not in DROP:
                    if j == 0:
                        nc.tensor.matmul(
                            s2v[:, :, 1:2, :], lhsT=ident,
                            rhs=bias_edge[:, 0:256],
                            start=False, stop=True, skip_group_check=True)
                    elif j == NBLK - 1:
                        nc.tensor.matmul(
                            s2v[:, :, 1:2, :], lhsT=ident,
                            rhs=bias_edge[:, 256:512],
                            start=False, stop=True, skip_group_check=True)
                    else:
                        nc.tensor.matmul(
                            s2v[:, :, 0:3:2, :], lhsT=ident, rhs=bias_int,
                            start=False, stop=True, skip_group_check=True)
                if "exp" not in DROP:
                    dst = pt[:, j * nh * 384 : (j + 1) * nh * 384].rearrange(
                        "p (u x) -> p u x", u=nh)
                    src = s2.rearrange("p (u x) -> p u x", u=2)[:, 0:nh, :]
                    flags = [on_dve() for _ in range(nh)]
                    if nh == 2 and flags[0] == flags[1]:
                        _emit_exp(nc, dst[:, :, lo:hi], src[:, :, lo:hi],
                                  flags[0])
                    else:
                        for u in range(nh):
                            _emit_exp(nc, dst[:, u:u+1, lo:hi],
                                      src[:, u:u+1, lo:hi], flags[u])
            else:
                # solo head: batch two consecutive key blocks per psum tile
                if j % 2 == 0:
                    s2_hold = pss.tile([128, 1024], F32, tag="s2",
                                       name=f"s2_{pr}_{j}")
                if "qk" not in DROP:
                    nc.tensor.matmul(
                        s2_hold[:, (j % 2) * 512 + lo : (j % 2) * 512 + hi],
                        lhsT=kt[0:64, j * 128:(j + 1) * 128],
                        rhs=qt[0:64, st + lo : st + hi],
                        start=True,
                        stop=False,
                        tile_position=(0, 0),
                        skip_group_check=True)
                if j % 2 == 1 and "mask" not in DROP:
                    s2v = s2_hold.rearrange("p (u s c) -> p u s c", u=2, s=4)
                    if j == 1:
                        nc.tensor.matmul(
                            s2v[:, 0:1, 1:2, :], lhsT=ident,
                            rhs=bias_edge[:, 0:128],
                            start=False, stop=False, skip_group_check=True)
                        nc.tensor.matmul(
                            s2v[:, 1:2, 0:3:2, :], lhsT=ident,
                            rhs=bias_int[:, 256:512],
                            start=False, stop=True, skip_group_check=True)
                    elif j == NBLK - 1:
                        nc.tensor.matmul(
                            s2v[:, 0:1, 0:3:2, :], lhsT=ident,
                            rhs=bias_int[:, 0:256],
                            start=False, stop=False, skip_group_check=True)
                        nc.tensor.matmul(
                            s2v[:, 1:2, 1:2, :], lhsT=ident,
                            rhs=bias_edge[:, 256:384],
                            start=False, stop=True, skip_group_check=True)
                    else:
                        nc.tensor.matmul(
                            s2v[:, :, 0:3:2, :], lhsT=ident, rhs=bias_int,
                            start=False, stop=True, skip_group_check=True)
                if j % 2 == 1 and "exp" not in DROP:
                    dst = pt[:, (j - 1) * 384 : (j + 1) * 384].rearrange(
                        "p (u x) -> p u x", u=2)
                    src = s2_hold.rearrange("p (u x) -> p u x", u=2)
                    flags = [on_dve() for _ in range(2)]
                    wins = [_exp_window(j - 1), _exp_window(j)]
                    if flags[0] == flags[1] and wins[0] == wins[1] == (0, 384):
                        _emit_exp(nc, dst, src[:, :, 0:384], flags[0])
                    else:
                        for jj_half in range(2):
                            l2, h2 = wins[jj_half]
                            _emit_exp(nc, dst[:, jj_half:jj_half+1, l2:h2],
                                      src[:, jj_half:jj_half+1, l2:h2],
                                      flags[jj_half])

            # PV + normalize per 4-j span (identical to baseline, minus masks)
            if j % 4 == 3:
                m4 = j // 4
                # defer PV by an extra group: the burst's first matmul then
                # waits on an exp ~7 tiles back (done), not ~3 (in flight),
                # removing head-of-line stalls on the in-order PE queue
                if m4 < 2:
                    groups = []
                elif m4 < NBLK // 4 - 1:
                    groups = [m4 - 2]
                else:
                    groups = [m4 - 2, m4 - 1, m4]
                for g in (groups if "pv" not in DROP else []):
                    for u in range(nh):
                        og = pso.tile([128, 260], F32, tag="og",
                                      name=f"og{pr}_{g}_{u}")
                        first = True
                        for m in range(4):
                            i = 4 * g + m
                            js = [jj for jj in (i - 1, i, i + 1)
                                  if 0 <= jj < NBLK]
                            for jj in js:
                                rel = _rel_slice(i, jj)
                                base = (jj * nh + u) * 384
                                nc.tensor.matmul(
                                    og[:, m * 65 : m * 65 + 65],
                                    lhsT=pt[:, base + rel * 128 : base + (rel + 1) * 128],
                                    rhs=vts[u][:, jj * 65 : (jj + 1) * 65],
                                    start=first,
                                    stop=(m == 3 and jj == js[-1]),
                                    skip_group_check=True,
                                )
                                first = False
                        ogv = og.rearrange("p (m c) -> p m c", m=4)
                        rg = rts[u][:, 4 * g : 4 * g + 4]
                        nc.vector.reciprocal(rg, ogv[:, :, 64])
                        osl = ots[u][:, 4 * g * 64 : (4 * g + 4) * 64].rearrange(
                            "p (m d) -> p m d", m=4)
                        nc.vector.tensor_tensor(
                            osl, ogv[:, :, 0:64],
                            rg[:, :, None].to_broadcast(osl.shape),
                            mybir.AluOpType.mult,
                        )
                        if g % 2 == 1:
                            h = hds[u] % n_heads
                            nc.sync.dma_start(
                                out=out[h][:, 4 * (g - 1):4 * (g + 1), :],
                                in_=ots[u][:, 4 * (g - 1) * 64:4 * (g + 1) * 64]
                                    .rearrange("p (n d) -> p n d", n=8),
                            )


def _emit_exp(nc, dst, src, use_dve: bool):
    if use_dve:
        nc.vector.tensor_scalar(dst.bitcast(I16), src, FEXP_A, FEXP_B,
                                mybir.AluOpType.mult, mybir.AluOpType.add)
    else:
        nc.scalar.activation(dst, src, EXP, bias=0.0, scale=0.125)


def make_consts() -> dict[str, np.ndarray]:
    w16 = np.eye(128, dtype=np.float16)

    kk = np.arange(128, dtype=np.int32)[:, None]   # partition = key
    xx = np.arange(128, dtype=np.int32)[None, :]   # col within slice
    slot0 = np.where(xx >= kk, 0.0, -BIAS_B).astype(np.float32)  # rel slice 0
    slot1 = np.where(xx <= kk, 0.0, -BIAS_B).astype(np.float32)  # rel slice 2

    # (u, s, c) order matching the strided psum out APs
    b_int = np.concatenate([slot0, slot1, slot0, slot1], axis=1).astype(np.float16)
    b_edge = np.concatenate([slot1, slot1, slot0, slot0], axis=1).astype(np.float16)
    return {"w8": w16, "bInt": b_int, "bEdge": b_edge}


_CACHE: dict = {}


def prepare_in_maps(q: np.ndarray, k: np.ndarray, v: np.ndarray) -> list[dict]:
    q = np.asarray(q, dtype=np.float32)
    k = np.asarray(k, dtype=np.float32)
    v = np.asarray(v, dtype=np.float32)

    qT = np.ascontiguousarray(
        q.reshape(B * H, L, D).transpose(0, 2, 1)
    ).astype(np.float16)
    kT = np.ascontiguousarray(
        k.reshape(B * H, L, D).transpose(0, 2, 1)
    ).astype(np.float16)
    vb = v.reshape(B * H, NBLK, 128, D).transpose(0, 2, 1, 3)
    vA = np.concatenate(
        [vb, np.ones((B * H, 128, NBLK, 1), np.float32)], axis=3
    ).astype(np.float16)
    vA = np.ascontiguousarray(vA)
    consts = make_consts()

    in_maps = []
    for c in range(NCORES):
        s = slice(c * HPC, (c + 1) * HPC)
        in_maps.append({"qT": qT[s], "kT": kT[s], "vA": vA[s], **consts})
    return in_maps


def kernel(q: np.ndarray, k: np.ndarray, v: np.ndarray) -> np.ndarray:
    from concourse.bass_utils import run_bass_kernel_spmd

    in_maps = prepare_in_maps(q, k, v)
    if "nc" not in _CACHE:
        _CACHE["nc"] = build_nc(HPC)
    nc = _CACHE["nc"]

    res = run_bass_kernel_spmd(nc, in_maps, list(range(NCORES)))
    outs = [res.results[c]["out"] for c in range(NCORES)]  # [3, 128, NBLK, 64]
    full = np.concatenate(outs, axis=0)          # [24, 128, NBLK, 64]
    full = full.transpose(0, 2, 1, 3).reshape(B, H, L, D)  # query-major
    return full.astype(np.float32)


if __name__ == "__main__":
    rng = np.random.default_rng(0)
    q = rng.standard_normal((B, H, L, D), dtype=np.float32)
    k = rng.standard_normal((B, H, L, D), dtype=np.float32)
    v = rng.standard_normal((B, H, L, D), dtype=np.float32)
    o = kernel(q, k, v)
    print("out", o.shape, o.dtype)
